# revision 62
# baseline (speedup 1.0000x reference)
"""GAT self-attention Trainium2 kernel (v2).

Full inputs -> shard graphs over 8 NeuronCores -> full output.

Math (per graph n, reference reformulated):
  g_i = sigmoid(relu(q @ W1_i) @ W2_i)            [2d]
  u_i^L = W_i @ (g_i[:d] * a_i[:d])               [k]   (left projector)
  u_i^R = W_i @ (g_i[d:] * a_i[d:])               [k]   (right projector)
  left_i = X @ u_i^L ; right_i = X @ u_i^R        [E]
  score[i,j] = lrelu(left_t[i] + right_t[j]), t = adj[i,j]; -BIG if adj==0
  Ex = exp(score); rs = rowsum(Ex)
  out = Ex^T @ (X @ W_2 / rs[:,None])             (== softmax(score)^T @ (X @ W_2))

Host staging: x/q/W transposed + bf16; adj one-hot masks as uint8.
Device layout trick: the LR matmul emits a 12-row block
  [L1, 1, L2, 1, L3, 1, 1, R1, 1, R2, 1, R3]
(ones rows filled by a rank-1 "needle" accumulation), so each type's
outer-sum score matmul reads its [L_t; 1] / [1; R_t] operand pair
directly -- no staging copies or DMAs.
"""
import numpy as np
from contextlib import ExitStack

import ml_dtypes

import concourse.bass as bass
import concourse.tile as tile
from concourse import mybir, bacc
from concourse.masks import make_identity

F32 = mybir.dt.float32
BF16 = mybir.dt.bfloat16
U8 = mybir.dt.uint8
AF = mybir.ActivationFunctionType
OP = mybir.AluOpType

N_CORES = 8
N, E, K, D = 64, 512, 512, 512   # graphs, entities, in_dim, out_dim
NG = N // N_CORES                # graphs per core
NT = 3                           # edge types
P = 128
EC = E // P                      # 4 partition chunks of E
KC = K // P
DC = D // P
TD2 = 2 * D
DC2 = TD2 // P                   # 8 chunks of the 2d gate dim
NEG_BIG = -200.0
LRELU_SLOPE = 0.2


def _dma_split(nc, dst, src, pieces):
    """Split a big load along dim 1 across sync/scalar queues."""
    n0 = dst.shape[1]
    step = max(1, n0 // pieces)
    engs = [nc.sync, nc.scalar]
    i = 0
    c = 0
    while i < n0:
        j = min(n0, i + step)
        engs[c % 2].dma_start(dst[:, i:j], src[:, i:j])
        i = j
        c += 1


def build(nc, reps=1):
    xT = nc.dram_tensor("xT", [NG, K, E], BF16, kind="ExternalInput").ap()
    masks = nc.dram_tensor("masks", [NG, 3, E, E], U8, kind="ExternalInput").ap()
    qT = nc.dram_tensor("qT", [K, NG], BF16, kind="ExternalInput").ap()
    at = nc.dram_tensor("at", [P, DC2, NT], F32, kind="ExternalInput").ap()
    WtT = nc.dram_tensor("WtT", [NT, D, K], BF16, kind="ExternalInput").ap()
    Wt2 = nc.dram_tensor("Wt2", [K, D], BF16, kind="ExternalInput").ap()
    W1 = nc.dram_tensor("W1", [NT, K, TD2], BF16, kind="ExternalInput").ap()
    W2q = nc.dram_tensor("W2q", [NT, TD2, TD2], BF16, kind="ExternalInput").ap()
    out = nc.dram_tensor("out", [NG, E, D], BF16, kind="ExternalOutput").ap()
    nc._gat_io = (xT, masks, qT, at, WtT, Wt2, W1, W2q, out)
    _build_once(nc, reps)


def _build_once(nc, reps=1):
    xT, masks, qT, at, WtT, Wt2, W1, W2q, out = nc._gat_io
    with tile.TileContext(nc) as tc, ExitStack() as ctx:
        # ---------------- persistent ----------------
        pers = ctx.enter_context(tc.tile_pool(name="pers", bufs=1))
        ident = pers.tile([P, P], F32)
        make_identity(nc, ident[:])
        ident_bf = pers.tile([P, P], BF16)
        nc.vector.tensor_copy(ident_bf[:], ident[:])
        # U_all[k%128, kc, c, n]: c=t -> u_t^L ; c=3+t -> u_t^R
        U_all = pers.tile([P, KC, 6, NG], BF16)
        qT_sb = pers.tile([P, KC, NG], BF16)
        nc.sync.dma_start(qT_sb[:], qT.rearrange("(c p) n -> p c n", p=P))
        at_sb = pers.tile([P, DC2, NT], F32)
        nc.scalar.dma_start(at_sb[:], at)
        # Persistent AB ring: ones rows at fixed spots, L/R rows DMA'd per
        # graph. AB[32t + q, 0:E] = [L_t; 1], AB[32t + q, E:2E] = [1; R_t].
        ones3 = pers.tile([NT, 1, E], BF16)
        nc.vector.memset(ones3[:], 1.0)
        AB_ring = [pers.tile([96, 2 * E], BF16, tag=f"ab{r}", name=f"ab{r}")
                   for r in range(5)]

        def fill_ab_ones():
            # engine ops need 32-aligned partition bases; odd-row ones are
            # written via DMA, which has no such restriction. Emitted after
            # prep so the startup HWDGE slots go to the weight stream.
            for ab in AB_ring:
                abg = ab.rearrange("(g q) e2 -> g q e2", q=32)
                nc.sync.dma_start(abg[:, 1:2, 0:E], ones3[:])
                nc.scalar.dma_start(abg[:, 0:1, E:2 * E], ones3[:])
        Wt2_sb = pers.tile([P, KC, D], BF16)

        # ---------------- pools ----------------
        sbuf = ctx.enter_context(tc.tile_pool(name="sbuf", bufs=5))
        perg = ctx.enter_context(tc.tile_pool(name="perg", bufs=NG))
        small = ctx.enter_context(tc.tile_pool(name="small", bufs=5))
        ps_v = ctx.enter_context(tc.tile_pool(name="ps_v", bufs=6, space="PSUM"))
        ps_big = ctx.enter_context(tc.tile_pool(name="ps_big", bufs=2, space="PSUM"))

        # ---------------- prep: gates -> U vectors ----------------
        # All gate matmuls use the NG(=8)-row operand as the 128-col-max
        # stationary side and stream the big weight as the moving side, so
        # each stage is a handful of ap=512 matmuls instead of dozens of
        # ap=8 ones; small [8, .] results are transposed back on the PE.
        # Emitted as a generator with a yield after each weight-bound stage
        # so prep-independent H2 blocks can be interleaved into the PE queue.
        prep = ctx.enter_context(tc.tile_pool(name="prep", bufs=1))

        def prep_type_stages(i):
            W1_sb = prep.tile([P, KC, TD2], BF16, tag="w1")
            src1 = W1[i].rearrange("(c p) f -> p c f", p=P)
            nc.sync.dma_start(W1_sb[:, :, 0:D], src1[:, :, 0:D])
            nc.scalar.dma_start(W1_sb[:, :, D:TD2], src1[:, :, D:TD2])
            # rr[n, o2] = relu(q @ W1_i), two 512-col halves
            rr_sb = prep.tile([NG, TD2], BF16, tag="rr")
            for h in range(2):
                pr = ps_big.tile([NG, D], F32, tag="big")
                for kc in range(KC):
                    nc.tensor.matmul(
                        pr[:], qT_sb[:, kc, :],
                        W1_sb[:, kc, h * D:(h + 1) * D],
                        start=(kc == 0), stop=(kc == KC - 1))
                nc.scalar.activation(rr_sb[:, h * D:(h + 1) * D], pr[:],
                                     AF.Relu)
            # rrT[o2%128, oc2, n] via PE transposes
            prT = ps_v.tile([P, DC2, NG], BF16, tag="v")
            for b in range(DC2):
                nc.tensor.transpose(prT[:, b, :],
                                    rr_sb[:, b * P:(b + 1) * P],
                                    ident_bf[:NG, :NG])
            rrT = prep.tile([P, DC2, NG], BF16, tag="rrT")
            nc.vector.tensor_copy(rrT[:], prT[:])
            yield
            W2_sb = prep.tile([P, DC2, TD2], BF16, tag="w2")
            src2 = W2q[i].rearrange("(c p) f -> p c f", p=P)
            for pc in range(4):
                dch, fh = pc % 2, pc // 2
                eng = nc.sync if pc % 2 == 0 else nc.scalar
                eng.dma_start(
                    W2_sb[:, dch * 4:(dch + 1) * 4, fh * D:(fh + 1) * D],
                    src2[:, dch * 4:(dch + 1) * 4, fh * D:(fh + 1) * D])
            # g[n, o2] = sigmoid(rr @ W2q_i)
            g_sb = prep.tile([NG, TD2], BF16, tag="g")
            for h in range(2):
                pg = ps_big.tile([NG, D], F32, tag="big")
                for dc in range(DC2):
                    nc.tensor.matmul(
                        pg[:], rrT[:, dc, :],
                        W2_sb[:, dc, h * D:(h + 1) * D],
                        start=(dc == 0), stop=(dc == DC2 - 1))
                nc.scalar.activation(g_sb[:, h * D:(h + 1) * D], pg[:],
                                     AF.Sigmoid)
            # vT[o2%128, dc, s, n] = g^T * a_i (a-mult fused into the
            # copy), (s, n) adjacent so both u-sides share one stationary
            pgT = ps_v.tile([P, DC2, NG], BF16, tag="v")
            for b in range(DC2):
                nc.tensor.transpose(pgT[:, b, :],
                                    g_sb[:, b * P:(b + 1) * P],
                                    ident_bf[:NG, :NG])
            vT = prep.tile([P, DC, 2, NG], BF16, tag="vT")
            for s in range(2):
                nc.vector.tensor_tensor(
                    vT[:, :, s, :], pgT[:, s * DC:(s + 1) * DC, :],
                    at_sb[:, s * DC:(s + 1) * DC, i:i + 1].broadcast_to(
                        (P, DC, NG)),
                    OP.mult)
            yield
            # u_i^{L,R}[n, k] = v-half @ W_i^T, both sides in one
            # 16-col stationary; transpose into U_all
            WtT_sb = prep.tile([P, DC, K], BF16, tag="wtt")
            _dma_split(nc, WtT_sb[:],
                       WtT[i].rearrange("(c p) k -> p c k", p=P), 2)
            pu = ps_big.tile([2 * NG, K], F32, tag="big")
            for dc in range(DC):
                nc.tensor.matmul(
                    pu[:], vT[:, dc, :, :], WtT_sb[:, dc, :],
                    start=(dc == 0), stop=(dc == DC - 1))
            u_sb = prep.tile([2 * NG, K], BF16, tag="u")
            nc.scalar.copy(u_sb[:], pu[:])
            puT = ps_v.tile([P, KC, 2 * NG], BF16, tag="v")
            for kc in range(KC):
                nc.tensor.transpose(puT[:, kc, :],
                                    u_sb[:, kc * P:(kc + 1) * P],
                                    ident_bf[:2 * NG, :2 * NG])
            nc.vector.tensor_copy(U_all[:, :, i, :], puT[:, :, 0:NG])
            nc.vector.tensor_copy(U_all[:, :, 3 + i, :], puT[:, :, NG:2 * NG])
            yield

        # ---------------- H2 = X @ W_2 (score-independent) ----------------
        xts = [None] * NG
        h2s = [None] * NG
        h2_engs = [nc.scalar]

        def h2_one(n):
            Xt_sb = perg.tile([P, KC, E], BF16, tag="X")
            nc.sync.dma_start(Xt_sb[:], xT[n].rearrange("(c p) e -> p c e", p=P))
            H2_sb = perg.tile([P, EC, D], BF16, tag="H2")
            for ic in range(EC):
                pH = ps_big.tile([P, D], F32, tag="big")
                for kc in range(KC):
                    nc.tensor.matmul(pH[:], Xt_sb[:, kc, ic * P:(ic + 1) * P],
                                     Wt2_sb[:, kc, :],
                                     start=(kc == 0), stop=(kc == KC - 1))
                eng = h2_engs[0]
                if eng is nc.scalar:
                    eng.copy(H2_sb[:, ic, :], pH[:])
                else:
                    eng.tensor_copy(H2_sb[:, ic, :], pH[:])
            xts[n] = Xt_sb
            h2s[n] = H2_sb

        # ---------------- main per-graph pipeline ----------------
        def phase1(n):
            """masks + the L/R rows + scattered outer-sum operands"""
            Xt_sb = xts[n]
            m_sb = sbuf.tile([P, 3, EC, E], U8, tag="m")
            nc.sync.dma_start(m_sb[:], masks[n].rearrange("m (c p) j -> p m c j", p=P))

            pLR = ps_big.tile([6, E], F32, tag="big")
            for kc in range(KC):
                nc.tensor.matmul(pLR[:], U_all[:, kc, :, n], Xt_sb[:, kc, :],
                                 start=(kc == 0), stop=(kc == KC - 1))
            LR_sb = small.tile([6, E], BF16, tag="lrs")
            nc.scalar.copy(LR_sb[:], pLR[:])
            # Scatter L/R rows to matmul-legal partition bases {0,32,64};
            # the ones rows are pre-set in the persistent ring tiles.
            AB = AB_ring[n % 5]
            ABg = AB.rearrange("(g q) e2 -> g q e2", q=32)
            nc.sync.dma_start(
                ABg[:, 0:1, 0:E],
                LR_sb[0:3].rearrange("(g q) e -> g q e", q=1))
            nc.sync.dma_start(
                ABg[:, 1:2, E:2 * E],
                LR_sb[3:6].rearrange("(g q) e -> g q e", q=1))
            return dict(Xt_sb=Xt_sb, m_sb=m_sb, AB=AB)

        def scores(n, st):
            """masked scores -> normalized exp matrix"""
            m_sb = st["m_sb"]; AB = st["AB"]
            E_sb = [sbuf.tile([P, E], BF16, tag=f"E{ic}", name=f"E{ic}") for ic in range(EC)]
            rs = [small.tile([P, 1], F32, tag=f"rs{ic}", name=f"rs{ic}") for ic in range(EC)]
            rsr = [small.tile([P, 1], F32, tag=f"rsr{ic}", name=f"rsr{ic}") for ic in range(EC)]
            for ic in range(EC):
                # merge tree: S3 into S2's bank first (frees it early), then
                # into S1 via the combined m2|m3 mask -- peak 2 live banks.
                pva = ps_v.tile([P, E], F32, tag="v")
                nc.tensor.matmul(
                    pva[:], AB[32:34, ic * P:(ic + 1) * P],
                    AB[32:34, E:2 * E], start=True, stop=True)
                pvb = ps_v.tile([P, E], F32, tag="v")
                nc.tensor.matmul(
                    pvb[:], AB[64:66, ic * P:(ic + 1) * P],
                    AB[64:66, E:2 * E], start=True, stop=True)
                nc.vector.copy_predicated(pva[:], m_sb[:, 2, ic, :], pvb[:])
                pvc = ps_v.tile([P, E], F32, tag="v")
                nc.tensor.matmul(
                    pvc[:], AB[0:2, ic * P:(ic + 1) * P],
                    AB[0:2, E:2 * E], start=True, stop=True)
                nc.vector.copy_predicated(pvc[:], m_sb[:, 1, ic, :], pva[:])
                # lrelu(x) = max(0.2x, x); |score| < 10 so exp of unmasked
                # garbage is finite -- adj==0 entries are zeroed by the mz
                # multiply below, which also yields the masked rowsum.
                # (GPSIMD cannot touch PSUM, so it gets the SBUF-side ops.)
                # exp(lrelu(x)) = max(exp(0.2x), exp(x)): two one-PSUM-read
                # Act exps (same table), then max+mask merge on Pool in SBUF
                e1 = small.tile([P, E], BF16, tag=f"e1{ic}", name=f"e1{ic}")
                nc.scalar.activation(e1[:], pvc[:], AF.Exp, scale=LRELU_SLOPE)
                nc.scalar.activation(E_sb[ic][:], pvc[:], AF.Exp)
                nc.vector.tensor_tensor(E_sb[ic][:], E_sb[ic][:], e1[:], OP.max)
                nc.vector.scalar_tensor_tensor(
                    E_sb[ic][:], m_sb[:, 0, ic, :], 1.0, E_sb[ic][:],
                    OP.mult, OP.mult, accum_out=rs[ic][:])
                nc.vector.reciprocal(rsr[ic][:], rs[ic][:])
                # normalize H2 rows instead of Ex -- decouples the out-matmul
                # from the Ex chain tail; split across DVE and Act
                if ic % 2 == 0:
                    nc.vector.tensor_scalar(h2s[n][:, ic, :], h2s[n][:, ic, :],
                                            rsr[ic][:], None, OP.mult)
                else:
                    nc.scalar.activation(h2s[n][:, ic, :], h2s[n][:, ic, :],
                                         AF.Copy, scale=rsr[ic][:])
            return E_sb

        def outp(n, E_sb):
            """out = Ex_norm^T @ H2"""
            H2s = h2s[n]
            for jc in range(EC):
                pO = ps_big.tile([P, D], F32, tag="big")
                for ic in range(EC):
                    nc.tensor.matmul(pO[:], E_sb[ic][:, jc * P:(jc + 1) * P],
                                     H2s[:, ic, :],
                                     start=(ic == 0), stop=(ic == EC - 1))
                o_sb = small.tile([P, D], BF16, tag="osb")
                nc.scalar.copy(o_sb[:], pO[:])
                nc.sync.dma_start(out[n, jc * P:(jc + 1) * P, :], o_sb[:])

        def body_all(_iv=None):
            # Interleave prep stages (weight-DMA bound) with H2 blocks
            # (prep-independent PE work) so the in-order PE queue never
            # stalls on a weight load.
            gens = [prep_type_stages(i) for i in range(NT)]
            next(gens[0])                   # W1[0] queued first
            _dma_split(nc, Wt2_sb[:], Wt2.rearrange("(c p) d -> p c d", p=P), 2)
            h2_one(0)
            next(gens[0]); h2_one(1)
            next(gens[0], None); h2_one(2)
            next(gens[1]); h2_one(3)
            next(gens[1]); h2_one(4)
            next(gens[1], None); h2_one(5)
            next(gens[2]); h2_one(6)
            next(gens[2]); h2_one(7)
            next(gens[2], None)
            fill_ab_ones()
            # depth-2 software pipeline: LR/scores of graph n+2 are emitted
            # ahead of out(n) so the in-order PE queue always has independent
            # matmuls while the elementwise score chain of a graph drains.
            es = {k: scores(k, phase1(k)) for k in range(4)}
            for n in range(NG):
                if n + 4 < NG:
                    es[n + 4] = scores(n + 4, phase1(n + 4))
                outp(n, es.pop(n))

        if reps == 1:
            body_all()
        else:
            with tc.For_i(0, reps, 1) as _iv:
                body_all(_iv)
    return nc


_NC_CACHE = {}
TRACE = False
_LAST = {}


def _get_nc():
    if "nc" not in _NC_CACHE:
        nc = bacc.Bacc("TRN2", target_bir_lowering=False, debug=False)
        build(nc)
        nc.compile()
        _NC_CACHE["nc"] = nc
    return _NC_CACHE["nc"]


def kernel(input_state, adj, entity_mask, query_vec, W_type, a_type,
           qattn_W1, qattn_W2):
    from concourse import bass_utils
    nc = _get_nc()
    bf = ml_dtypes.bfloat16
    input_state = np.asarray(input_state, dtype=np.float32)
    adj = np.asarray(adj, dtype=np.int32)
    query_vec = np.asarray(query_vec, dtype=np.float32)

    xT_all = np.ascontiguousarray(
        input_state.transpose(0, 2, 1)).astype(bf)              # [N, K, E]
    masks_all = np.ascontiguousarray(np.stack(
        [(adj != 0), (adj == 2) | (adj == 3), (adj == 3)], axis=1)).astype(np.uint8)
    qT_all = np.ascontiguousarray(query_vec.T).astype(bf)       # [K, N]
    at_h = np.ascontiguousarray(
        np.asarray(a_type, np.float32).reshape(NT, DC2, P).transpose(2, 1, 0))
    WtT_h = np.ascontiguousarray(
        np.asarray(W_type, np.float32).transpose(0, 2, 1)).astype(bf)
    Wt2_h = np.ascontiguousarray(np.asarray(W_type, np.float32)[2]).astype(bf)
    W1_h = np.ascontiguousarray(np.asarray(qattn_W1, np.float32)).astype(bf)
    W2q_h = np.ascontiguousarray(np.asarray(qattn_W2, np.float32)).astype(bf)

    in_maps = []
    for c in range(N_CORES):
        sl = slice(c * NG, (c + 1) * NG)
        in_maps.append({
            "xT": xT_all[sl], "masks": masks_all[sl],
            "qT": np.ascontiguousarray(qT_all[:, sl]),
            "at": at_h, "WtT": WtT_h, "Wt2": Wt2_h,
            "W1": W1_h, "W2q": W2q_h,
        })
    res = bass_utils.run_bass_kernel_spmd(nc, in_maps, core_ids=list(range(N_CORES)),
                                          trace=TRACE, stitch_traces=TRACE)
    _LAST["exec_ns"] = res.exec_time_ns
    _LAST["mean_ns"] = res.mean_exec_time_ns
    _LAST["trace"] = res.instructions_and_trace
    _LAST["scope_times"] = res.per_core_scope_times
    out = np.concatenate([np.asarray(r["out"]) for r in res.results], axis=0)
    return out.astype(np.float32)


# revision 63
# speedup vs baseline: 1.0004x; 1.0004x over previous
"""GAT self-attention Trainium2 kernel (v2).

Full inputs -> shard graphs over 8 NeuronCores -> full output.

Math (per graph n, reference reformulated):
  g_i = sigmoid(relu(q @ W1_i) @ W2_i)            [2d]
  u_i^L = W_i @ (g_i[:d] * a_i[:d])               [k]   (left projector)
  u_i^R = W_i @ (g_i[d:] * a_i[d:])               [k]   (right projector)
  left_i = X @ u_i^L ; right_i = X @ u_i^R        [E]
  score[i,j] = lrelu(left_t[i] + right_t[j]), t = adj[i,j]; -BIG if adj==0
  Ex = exp(score); rs = rowsum(Ex)
  out = Ex^T @ (X @ W_2 / rs[:,None])             (== softmax(score)^T @ (X @ W_2))

Host staging: x/q/W transposed + bf16; adj shipped as uint8 one-hot
masks (mz=adj!=0, m23=adj in {2,3}, m3=adj==3).

Device structure:
 - prep streams the gate weights through the PE with the 8 queries as
   the stationary side, interleaved with per-graph H2 = X @ W_2 blocks
   so the in-order PE queue is never idle on a weight DMA;
 - per graph, one [6, E] LR matmul + a single scatter DMA place each
   type's outer-sum operand pair [L_t; 1] / [1; R_t] at the
   matmul-legal partition bases {0, 32, 64} of a persistent ring tile
   whose `ones` rows are written once;
 - scores: three rank-2 outer-sum matmuls -> copy_predicated merge ->
   exp(lrelu(x)) as max(exp(0.2x), exp(x)) (single-PSUM-read rule) ->
   mask multiply with rowsum accumulation -> H2 rows scaled by 1/rs;
 - out = Ex^T @ H2s, emitted as a depth-4 software pipeline.
"""
import numpy as np
from contextlib import ExitStack

import ml_dtypes

import concourse.bass as bass
import concourse.tile as tile
from concourse import mybir, bacc
from concourse.masks import make_identity

F32 = mybir.dt.float32
BF16 = mybir.dt.bfloat16
U8 = mybir.dt.uint8
AF = mybir.ActivationFunctionType
OP = mybir.AluOpType

N_CORES = 8
N, E, K, D = 64, 512, 512, 512   # graphs, entities, in_dim, out_dim
NG = N // N_CORES                # graphs per core
NT = 3                           # edge types
P = 128
EC = E // P                      # 4 partition chunks of E
KC = K // P
DC = D // P
TD2 = 2 * D
DC2 = TD2 // P                   # 8 chunks of the 2d gate dim
NEG_BIG = -200.0
LRELU_SLOPE = 0.2


def _dma_split(nc, dst, src, pieces):
    """Split a big load along dim 1 across sync/scalar queues."""
    n0 = dst.shape[1]
    step = max(1, n0 // pieces)
    engs = [nc.sync, nc.scalar]
    i = 0
    c = 0
    while i < n0:
        j = min(n0, i + step)
        engs[c % 2].dma_start(dst[:, i:j], src[:, i:j])
        i = j
        c += 1


def build(nc, reps=1):
    xT = nc.dram_tensor("xT", [NG, K, E], BF16, kind="ExternalInput").ap()
    masks = nc.dram_tensor("masks", [NG, 3, E, E], U8, kind="ExternalInput").ap()
    qT = nc.dram_tensor("qT", [K, NG], BF16, kind="ExternalInput").ap()
    at = nc.dram_tensor("at", [P, DC2, NT], F32, kind="ExternalInput").ap()
    WtT = nc.dram_tensor("WtT", [NT, D, K], BF16, kind="ExternalInput").ap()
    Wt2 = nc.dram_tensor("Wt2", [K, D], BF16, kind="ExternalInput").ap()
    W1 = nc.dram_tensor("W1", [NT, K, TD2], BF16, kind="ExternalInput").ap()
    W2q = nc.dram_tensor("W2q", [NT, TD2, TD2], BF16, kind="ExternalInput").ap()
    out = nc.dram_tensor("out", [NG, E, D], BF16, kind="ExternalOutput").ap()
    nc._gat_io = (xT, masks, qT, at, WtT, Wt2, W1, W2q, out)
    _build_once(nc, reps)


def _build_once(nc, reps=1):
    xT, masks, qT, at, WtT, Wt2, W1, W2q, out = nc._gat_io
    with tile.TileContext(nc) as tc, ExitStack() as ctx:
        # ---------------- persistent ----------------
        pers = ctx.enter_context(tc.tile_pool(name="pers", bufs=1))
        ident = pers.tile([P, P], F32)
        make_identity(nc, ident[:])
        ident_bf = pers.tile([P, P], BF16)
        nc.vector.tensor_copy(ident_bf[:], ident[:])
        # U_all[k%128, kc, c, n]: c=t -> u_t^L ; c=3+t -> u_t^R
        U_all = pers.tile([P, KC, 6, NG], BF16)
        qT_sb = pers.tile([P, KC, NG], BF16)
        nc.sync.dma_start(qT_sb[:], qT.rearrange("(c p) n -> p c n", p=P))
        at_sb = pers.tile([P, DC2, NT], F32)
        nc.scalar.dma_start(at_sb[:], at)
        # Persistent AB ring: ones rows at fixed spots, L/R rows DMA'd per
        # graph. AB[32t + q, 0:E] = [L_t; 1], AB[32t + q, E:2E] = [1; R_t].
        ones3 = pers.tile([NT, 1, E], BF16)
        nc.vector.memset(ones3[:], 1.0)
        AB_ring = [pers.tile([96, 2 * E], BF16, tag=f"ab{r}", name=f"ab{r}")
                   for r in range(5)]

        def fill_ab_ones():
            # engine ops need 32-aligned partition bases; odd-row ones are
            # written via DMA, which has no such restriction. Emitted after
            # prep so the startup HWDGE slots go to the weight stream.
            for ab in AB_ring:
                abg = ab.rearrange("(g q) e2 -> g q e2", q=32)
                nc.sync.dma_start(abg[:, 1:2, 0:E], ones3[:])
                nc.scalar.dma_start(abg[:, 0:1, E:2 * E], ones3[:])
        Wt2_sb = pers.tile([P, KC, D], BF16)

        # ---------------- pools ----------------
        sbuf = ctx.enter_context(tc.tile_pool(name="sbuf", bufs=4))
        perg = ctx.enter_context(tc.tile_pool(name="perg", bufs=NG))
        small = ctx.enter_context(tc.tile_pool(name="small", bufs=4))
        ps_v = ctx.enter_context(tc.tile_pool(name="ps_v", bufs=6, space="PSUM"))
        ps_big = ctx.enter_context(tc.tile_pool(name="ps_big", bufs=2, space="PSUM"))

        # ---------------- prep: gates -> U vectors ----------------
        # All gate matmuls use the NG(=8)-row operand as the 128-col-max
        # stationary side and stream the big weight as the moving side, so
        # each stage is a handful of ap=512 matmuls instead of dozens of
        # ap=8 ones; small [8, .] results are transposed back on the PE.
        # Emitted as a generator with a yield after each weight-bound stage
        # so prep-independent H2 blocks can be interleaved into the PE queue.
        prep = ctx.enter_context(tc.tile_pool(name="prep", bufs=1))

        def prep_type_stages(i):
            W1_sb = prep.tile([P, KC, TD2], BF16, tag="w1")
            src1 = W1[i].rearrange("(c p) f -> p c f", p=P)
            nc.sync.dma_start(W1_sb[:, :, 0:D], src1[:, :, 0:D])
            nc.scalar.dma_start(W1_sb[:, :, D:TD2], src1[:, :, D:TD2])
            # rr[n, o2] = relu(q @ W1_i), two 512-col halves
            rr_sb = prep.tile([NG, TD2], BF16, tag="rr")
            for h in range(2):
                pr = ps_big.tile([NG, D], F32, tag="big")
                for kc in range(KC):
                    nc.tensor.matmul(
                        pr[:], qT_sb[:, kc, :],
                        W1_sb[:, kc, h * D:(h + 1) * D],
                        start=(kc == 0), stop=(kc == KC - 1))
                nc.scalar.activation(rr_sb[:, h * D:(h + 1) * D], pr[:],
                                     AF.Relu)
            # rrT[o2%128, oc2, n] via PE transposes
            prT = ps_v.tile([P, DC2, NG], BF16, tag="v")
            for b in range(DC2):
                nc.tensor.transpose(prT[:, b, :],
                                    rr_sb[:, b * P:(b + 1) * P],
                                    ident_bf[:NG, :NG])
            rrT = prep.tile([P, DC2, NG], BF16, tag="rrT")
            nc.vector.tensor_copy(rrT[:], prT[:])
            yield
            W2_sb = prep.tile([P, DC2, TD2], BF16, tag="w2")
            src2 = W2q[i].rearrange("(c p) f -> p c f", p=P)
            for pc in range(4):
                dch, fh = pc % 2, pc // 2
                eng = nc.sync if pc % 2 == 0 else nc.scalar
                eng.dma_start(
                    W2_sb[:, dch * 4:(dch + 1) * 4, fh * D:(fh + 1) * D],
                    src2[:, dch * 4:(dch + 1) * 4, fh * D:(fh + 1) * D])
            # g[n, o2] = sigmoid(rr @ W2q_i)
            g_sb = prep.tile([NG, TD2], BF16, tag="g")
            for h in range(2):
                pg = ps_big.tile([NG, D], F32, tag="big")
                for dc in range(DC2):
                    nc.tensor.matmul(
                        pg[:], rrT[:, dc, :],
                        W2_sb[:, dc, h * D:(h + 1) * D],
                        start=(dc == 0), stop=(dc == DC2 - 1))
                nc.scalar.activation(g_sb[:, h * D:(h + 1) * D], pg[:],
                                     AF.Sigmoid)
            # vT[o2%128, dc, s, n] = g^T * a_i (a-mult fused into the
            # copy), (s, n) adjacent so both u-sides share one stationary
            pgT = ps_v.tile([P, DC2, NG], BF16, tag="v")
            for b in range(DC2):
                nc.tensor.transpose(pgT[:, b, :],
                                    g_sb[:, b * P:(b + 1) * P],
                                    ident_bf[:NG, :NG])
            vT = prep.tile([P, DC, 2, NG], BF16, tag="vT")
            for s in range(2):
                nc.vector.tensor_tensor(
                    vT[:, :, s, :], pgT[:, s * DC:(s + 1) * DC, :],
                    at_sb[:, s * DC:(s + 1) * DC, i:i + 1].broadcast_to(
                        (P, DC, NG)),
                    OP.mult)
            yield
            # u_i^{L,R}[n, k] = v-half @ W_i^T, both sides in one
            # 16-col stationary; transpose into U_all
            WtT_sb = prep.tile([P, DC, K], BF16, tag="wtt")
            _dma_split(nc, WtT_sb[:],
                       WtT[i].rearrange("(c p) k -> p c k", p=P), 2)
            pu = ps_big.tile([2 * NG, K], F32, tag="big")
            for dc in range(DC):
                nc.tensor.matmul(
                    pu[:], vT[:, dc, :, :], WtT_sb[:, dc, :],
                    start=(dc == 0), stop=(dc == DC - 1))
            u_sb = prep.tile([2 * NG, K], BF16, tag="u")
            nc.scalar.copy(u_sb[:], pu[:])
            puT = ps_v.tile([P, KC, 2 * NG], BF16, tag="v")
            for kc in range(KC):
                nc.tensor.transpose(puT[:, kc, :],
                                    u_sb[:, kc * P:(kc + 1) * P],
                                    ident_bf[:2 * NG, :2 * NG])
            nc.vector.tensor_copy(U_all[:, :, i, :], puT[:, :, 0:NG])
            nc.vector.tensor_copy(U_all[:, :, 3 + i, :], puT[:, :, NG:2 * NG])
            yield

        # ---------------- H2 = X @ W_2 (score-independent) ----------------
        xts = [None] * NG
        h2s = [None] * NG
        h2_engs = [nc.scalar]

        def h2_one(n):
            Xt_sb = perg.tile([P, KC, E], BF16, tag="X")
            nc.sync.dma_start(Xt_sb[:], xT[n].rearrange("(c p) e -> p c e", p=P))
            H2_sb = perg.tile([P, EC, D], BF16, tag="H2")
            for ic in range(EC):
                pH = ps_big.tile([P, D], F32, tag="big")
                for kc in range(KC):
                    nc.tensor.matmul(pH[:], Xt_sb[:, kc, ic * P:(ic + 1) * P],
                                     Wt2_sb[:, kc, :],
                                     start=(kc == 0), stop=(kc == KC - 1))
                eng = h2_engs[0]
                if eng is nc.scalar:
                    eng.copy(H2_sb[:, ic, :], pH[:])
                else:
                    eng.tensor_copy(H2_sb[:, ic, :], pH[:])
            xts[n] = Xt_sb
            h2s[n] = H2_sb

        # ---------------- main per-graph pipeline ----------------
        def phase1(n):
            """masks + the L/R rows + scattered outer-sum operands"""
            Xt_sb = xts[n]
            m_sb = sbuf.tile([P, 3, EC, E], U8, tag="m")
            nc.sync.dma_start(m_sb[:], masks[n].rearrange("m (c p) j -> p m c j", p=P))

            pLR = ps_big.tile([6, E], F32, tag="big")
            for kc in range(KC):
                nc.tensor.matmul(pLR[:], U_all[:, kc, :, n], Xt_sb[:, kc, :],
                                 start=(kc == 0), stop=(kc == KC - 1))
            LR_sb = small.tile([6, E], BF16, tag="lrs")
            nc.scalar.copy(LR_sb[:], pLR[:])
            # Scatter L/R rows to matmul-legal partition bases {0,32,64};
            # the ones rows are pre-set in the persistent ring tiles.
            AB = AB_ring[n % 5]
            ABg = AB.rearrange("(g q) e2 -> g q e2", q=32)
            nc.sync.dma_start(
                ABg[:, 0:1, 0:E],
                LR_sb[0:3].rearrange("(g q) e -> g q e", q=1))
            nc.sync.dma_start(
                ABg[:, 1:2, E:2 * E],
                LR_sb[3:6].rearrange("(g q) e -> g q e", q=1))
            return dict(Xt_sb=Xt_sb, m_sb=m_sb, AB=AB)

        def scores(n, st):
            """masked scores -> normalized exp matrix"""
            m_sb = st["m_sb"]; AB = st["AB"]
            E_sb = [sbuf.tile([P, E], BF16, tag=f"E{ic}", name=f"E{ic}") for ic in range(EC)]
            rs = [small.tile([P, 1], F32, tag=f"rs{ic}", name=f"rs{ic}") for ic in range(EC)]
            rsr = [small.tile([P, 1], F32, tag=f"rsr{ic}", name=f"rsr{ic}") for ic in range(EC)]
            for ic in range(EC):
                # merge tree: S3 into S2's bank first (frees it early), then
                # into S1 via the combined m2|m3 mask -- peak 2 live banks.
                pva = ps_v.tile([P, E], F32, tag="v")
                nc.tensor.matmul(
                    pva[:], AB[32:34, ic * P:(ic + 1) * P],
                    AB[32:34, E:2 * E], start=True, stop=True)
                pvb = ps_v.tile([P, E], F32, tag="v")
                nc.tensor.matmul(
                    pvb[:], AB[64:66, ic * P:(ic + 1) * P],
                    AB[64:66, E:2 * E], start=True, stop=True)
                nc.vector.copy_predicated(pva[:], m_sb[:, 2, ic, :], pvb[:])
                pvc = ps_v.tile([P, E], F32, tag="v")
                nc.tensor.matmul(
                    pvc[:], AB[0:2, ic * P:(ic + 1) * P],
                    AB[0:2, E:2 * E], start=True, stop=True)
                nc.vector.copy_predicated(pvc[:], m_sb[:, 1, ic, :], pva[:])
                # lrelu(x) = max(0.2x, x); |score| < 10 so exp of unmasked
                # garbage is finite -- adj==0 entries are zeroed by the mz
                # multiply below, which also yields the masked rowsum.
                # (GPSIMD cannot touch PSUM, so it gets the SBUF-side ops.)
                # exp(lrelu(x)) = max(exp(0.2x), exp(x)): two one-PSUM-read
                # Act exps (same table), then max+mask merge on Pool in SBUF
                e1 = small.tile([P, E], BF16, tag=f"e1{ic}", name=f"e1{ic}")
                nc.scalar.activation(e1[:], pvc[:], AF.Exp, scale=LRELU_SLOPE)
                nc.scalar.activation(E_sb[ic][:], pvc[:], AF.Exp)
                nc.vector.tensor_tensor(E_sb[ic][:], E_sb[ic][:], e1[:], OP.max)
                nc.vector.scalar_tensor_tensor(
                    E_sb[ic][:], m_sb[:, 0, ic, :], 1.0, E_sb[ic][:],
                    OP.mult, OP.mult, accum_out=rs[ic][:])
                nc.vector.reciprocal(rsr[ic][:], rs[ic][:])
                # normalize H2 rows instead of Ex -- decouples the out-matmul
                # from the Ex chain tail; split across DVE and Act
                if ic % 2 == 0:
                    nc.vector.tensor_scalar(h2s[n][:, ic, :], h2s[n][:, ic, :],
                                            rsr[ic][:], None, OP.mult)
                else:
                    nc.scalar.activation(h2s[n][:, ic, :], h2s[n][:, ic, :],
                                         AF.Copy, scale=rsr[ic][:])
            return E_sb

        def outp(n, E_sb):
            """out = Ex_norm^T @ H2"""
            H2s = h2s[n]
            for jc in range(EC):
                pO = ps_big.tile([P, D], F32, tag="big")
                for ic in range(EC):
                    nc.tensor.matmul(pO[:], E_sb[ic][:, jc * P:(jc + 1) * P],
                                     H2s[:, ic, :],
                                     start=(ic == 0), stop=(ic == EC - 1))
                o_sb = small.tile([P, D], BF16, tag="osb")
                nc.scalar.copy(o_sb[:], pO[:])
                nc.sync.dma_start(out[n, jc * P:(jc + 1) * P, :], o_sb[:])

        def body_all(_iv=None):
            # Interleave prep stages (weight-DMA bound) with H2 blocks
            # (prep-independent PE work) so the in-order PE queue never
            # stalls on a weight load.
            gens = [prep_type_stages(i) for i in range(NT)]
            next(gens[0])                   # W1[0] queued first
            _dma_split(nc, Wt2_sb[:], Wt2.rearrange("(c p) d -> p c d", p=P), 2)
            h2_one(0)
            next(gens[0]); h2_one(1)
            next(gens[0], None); h2_one(2)
            next(gens[1]); h2_one(3)
            next(gens[1]); h2_one(4)
            next(gens[1], None); h2_one(5)
            next(gens[2]); h2_one(6)
            next(gens[2]); h2_one(7)
            next(gens[2], None)
            fill_ab_ones()
            # depth-2 software pipeline: LR/scores of graph n+2 are emitted
            # ahead of out(n) so the in-order PE queue always has independent
            # matmuls while the elementwise score chain of a graph drains.
            es = {k: scores(k, phase1(k)) for k in range(4)}
            for n in range(NG):
                if n + 4 < NG:
                    es[n + 4] = scores(n + 4, phase1(n + 4))
                outp(n, es.pop(n))

        if reps == 1:
            body_all()
        else:
            with tc.For_i(0, reps, 1) as _iv:
                body_all(_iv)
    return nc


_NC_CACHE = {}
TRACE = False
_LAST = {}


def _get_nc():
    if "nc" not in _NC_CACHE:
        nc = bacc.Bacc("TRN2", target_bir_lowering=False, debug=False)
        build(nc)
        nc.compile()
        _NC_CACHE["nc"] = nc
    return _NC_CACHE["nc"]


def kernel(input_state, adj, entity_mask, query_vec, W_type, a_type,
           qattn_W1, qattn_W2):
    from concourse import bass_utils
    nc = _get_nc()
    bf = ml_dtypes.bfloat16
    input_state = np.asarray(input_state, dtype=np.float32)
    adj = np.asarray(adj, dtype=np.int32)
    query_vec = np.asarray(query_vec, dtype=np.float32)

    xT_all = np.ascontiguousarray(
        input_state.transpose(0, 2, 1)).astype(bf)              # [N, K, E]
    masks_all = np.ascontiguousarray(np.stack(
        [(adj != 0), (adj == 2) | (adj == 3), (adj == 3)], axis=1)).astype(np.uint8)
    qT_all = np.ascontiguousarray(query_vec.T).astype(bf)       # [K, N]
    at_h = np.ascontiguousarray(
        np.asarray(a_type, np.float32).reshape(NT, DC2, P).transpose(2, 1, 0))
    WtT_h = np.ascontiguousarray(
        np.asarray(W_type, np.float32).transpose(0, 2, 1)).astype(bf)
    Wt2_h = np.ascontiguousarray(np.asarray(W_type, np.float32)[2]).astype(bf)
    W1_h = np.ascontiguousarray(np.asarray(qattn_W1, np.float32)).astype(bf)
    W2q_h = np.ascontiguousarray(np.asarray(qattn_W2, np.float32)).astype(bf)

    in_maps = []
    for c in range(N_CORES):
        sl = slice(c * NG, (c + 1) * NG)
        in_maps.append({
            "xT": xT_all[sl], "masks": masks_all[sl],
            "qT": np.ascontiguousarray(qT_all[:, sl]),
            "at": at_h, "WtT": WtT_h, "Wt2": Wt2_h,
            "W1": W1_h, "W2q": W2q_h,
        })
    res = bass_utils.run_bass_kernel_spmd(nc, in_maps, core_ids=list(range(N_CORES)),
                                          trace=TRACE, stitch_traces=TRACE)
    _LAST["exec_ns"] = res.exec_time_ns
    _LAST["mean_ns"] = res.mean_exec_time_ns
    _LAST["trace"] = res.instructions_and_trace
    _LAST["scope_times"] = res.per_core_scope_times
    out = np.concatenate([np.asarray(r["out"]) for r in res.results], axis=0)
    return out.astype(np.float32)
